# revision 1
# baseline (speedup 1.0000x reference)
"""Trainium2 Bass kernel for the DiffKS pipeline:
  x = invert_lpc(y, A_exc)         (order-6 time-varying FIR)
  out = sample_wise_lpc(x, A_loop) (order-2 time-varying all-pole IIR)

Sharding: pure data-parallel over batch B=48 -> 6 rows per core x 8 cores.

Per-core algorithm (all fp32, VectorE-centric):
  * Time axis (T=88200, padded to 128*690) is chunked across the 128 SBUF
    partitions; every chunk re-runs the recurrence from 64 samples early
    ("warmup") with zero initial state. |A_loop| <= 0.25 makes the
    homogeneous dynamics contract by >=2x per 2 samples, so the wrong
    boundary state is attenuated below ~2^-32 by the time the chunk's
    real samples start - well below fp32 noise.
  * The FIR is 12 tensor_tensor passes (6 mult + 6 add) over the chunked
    layout.
  * The order-2 IIR is solved by pair-condensation into coupled order-1
    recurrences over (even,odd) sample pairs and Gauss-Seidel sweeps where
    each half-sweep is an exact first-order solve via the hardware
    tensor_tensor_scan. Worst-case contraction per sweep is
    (0.25/(1-0.25)) * (0.0625/(1-0.3125)) ~ 0.03, so NSWEEP=3 sweeps +
    a final even half-sweep leave iteration error below fp32 rounding
    noise (measured 3.2e-7 relative vs the sequential reference).
"""

import numpy as np

import sys

for _p in ("/opt/trn_rl_repo",):
    if _p not in sys.path:
        sys.path.insert(0, _p)

from concourse import bacc, bass, mybir, tile
from concourse.bass_utils import run_bass_kernel_spmd

B, T = 48, 88200
NCORES = 8
BLOC = B // NCORES       # 6 batch rows per core
K, L = 128, 690          # chunks x chunk length; K*L = 88320 >= T
W = 64                   # warmup samples per chunk (must be even)
SEG = W + L              # 754 samples per chunk-segment
HP = SEG // 2            # 377 pairs per segment
PRE = 72                 # zeros prepended to every padded row
TP = PRE + K * L + 8     # 88400 padded row length
import os

NB = int(os.environ.get("KS_NB", "1"))       # batch rows per slab
NSLAB = BLOC // NB
NSWEEP = int(os.environ.get("KS_NSWEEP", "3"))  # Gauss-Seidel sweeps (incl. sweep 0)
# NOTE on GpSimd(Pool) offload: a Pool-instruction-heavy program (40+ pool
# ops, fine-grained DVE interleave) reproducibly hangs the exec unit on this
# HW path (NRT_EXEC_UNIT_UNRECOVERABLE) even though small Pool probes pass.
# The shipping config uses the WIDE layout with only 15 coarse pool ops
# (3 FIR tap multiplies + 2 adds per slab, one cross-engine edge per slab),
# which runs clean on HW and was validated at rel err 3.1e-7.
POOL_TAPS = int(os.environ.get("KS_POOL_TAPS", "4"))  # FIR taps on GpSimd (taps 7-PT..6)
POOL_EF = int(os.environ.get("KS_POOL_EF", "0"))      # E-setup + f2 on GpSimd
POOL_U = int(os.environ.get("KS_POOL_U", "0"))        # 0: none, 1: u2 on pool, 2: u1+u2 on pool
FINAL_HALF = int(os.environ.get("KS_FINAL", "1"))     # extra even half-sweep at the end
WIDE = int(os.environ.get("KS_WIDE", "1"))            # K=64,L=1380, 2 rows packed in partitions
POOL_LOW = int(os.environ.get("KS_POOL_LOW", "0"))    # pool owns taps 1..PT (DMAs land first)

MULT = mybir.AluOpType.mult
ADD = mybir.AluOpType.add

_compiled = {}


def _dram_view(handle, offset, dims):
    """Raw strided view of a DRAM tensor: dims = [(stride, count), ...]."""
    return bass.AP(handle, offset, [[s, c] for (s, c) in dims])


def _build_program_wide():
    """K=64 chunks x L=1380; partitions hold (row, chunk) = 2 rows per slab.

    Same algorithm as _build_program but with 2x longer instructions (less
    per-op overhead) and half the warmup fraction. All tiles are plain 2D
    [128, n]; partition p = row*64 + chunk.
    """
    Kw, Lw = 64, 1380
    SEGw = W + Lw            # 1444
    HPw = SEGw // 2          # 722
    nc = bacc.Bacc("TRN2", target_bir_lowering=False, debug=False)

    y_d = nc.dram_tensor("y_pad", (BLOC, TP), mybir.dt.float32, kind="ExternalInput")
    a_d = nc.dram_tensor("a_tap", (6, BLOC, TP), mybir.dt.float32, kind="ExternalInput")
    b1_d = nc.dram_tensor("b1_pad", (BLOC, TP), mybir.dt.float32, kind="ExternalInput")
    b2_d = nc.dram_tensor("b2_pad", (BLOC, TP), mybir.dt.float32, kind="ExternalInput")
    out_d = nc.dram_tensor("y_out", (BLOC, Kw * Lw), mybir.dt.float32, kind="ExternalOutput")

    v = nc.vector
    g = nc.gpsimd
    f32 = mybir.dt.float32

    def pair2(ap2, sel):
        n = ap2.shape[1]
        assert n % 2 == 0
        return ap2.rearrange("p (m two) -> p two m", two=2)[:, sel, :]

    with tile.TileContext(nc) as tc:
        with tc.tile_pool(name="main", bufs=2) as pool:
            for s in range(3):
                rows = [s * 2, s * 2 + 1]

                yt = pool.tile([128, SEGw + 8], f32, name=f"yt{s}", tag="yt")
                at = [pool.tile([128, SEGw], f32, name=f"at{k}_{s}", tag=f"at{k}") for k in range(1, 7)]
                b1t = pool.tile([128, SEGw], f32, name=f"b1t{s}", tag="b1t")
                b2t = pool.tile([128, SEGw], f32, name=f"b2t{s}", tag="b2t")
                xt = pool.tile([128, SEGw], f32, name=f"xt{s}", tag="xt")
                tmp = pool.tile([128, SEGw], f32, name=f"tmp{s}", tag="tmp")
                if POOL_TAPS > 0:
                    pp = pool.tile([128, SEGw], f32, name=f"pp{s}", tag="pp")
                    ptmp = pool.tile([128, SEGw], f32, name=f"ptmp{s}", tag="ptmp")
                e10 = pool.tile([128, HPw], f32, name=f"e10_{s}", tag="e10")
                e11 = pool.tile([128, HPw], f32, name=f"e11_{s}", tag="e11")
                f2 = pool.tile([128, HPw], f32, name=f"f2_{s}", tag="f2")
                u1 = pool.tile([128, HPw], f32, name=f"u1_{s}", tag="u1")
                u2 = pool.tile([128, HPw], f32, name=f"u2_{s}", tag="u2")
                s1 = pool.tile([128, HPw + 1], f32, name=f"s1_{s}", tag="s1")
                s2 = pool.tile([128, HPw + 1], f32, name=f"s2_{s}", tag="s2")
                yo = pool.tile([128, SEGw + 2], f32, name=f"yo{s}", tag="yo")

                # ---- input DMAs: one 128-partition transfer per tile
                # (DRAM side walks row x chunk x j; SBUF partition = row*64+chunk)
                # Order: yt, then the pool-owned taps, then b1/b2 (DVE E-setup),
                # then the DVE-owned taps - so both engines start ASAP.
                r0 = rows[0]
                pool_taps = list(range(1, POOL_TAPS + 1)) if POOL_LOW else list(range(7 - POOL_TAPS, 7))
                dve_taps = [k for k in range(1, 7) if k not in pool_taps]
                nc.sync.dma_start(
                    yt[:, :],
                    _dram_view(y_d, r0 * TP + 2, [(TP, 2), (Lw, Kw), (1, SEGw + 8)]),
                )
                for k in pool_taps + [0, -1] + dve_taps:
                    if k == 0:
                        nc.sync.dma_start(
                            b1t[:, :], _dram_view(b1_d, r0 * TP + 8, [(TP, 2), (Lw, Kw), (1, SEGw)])
                        )
                    elif k == -1:
                        nc.sync.dma_start(
                            b2t[:, :], _dram_view(b2_d, r0 * TP + 8, [(TP, 2), (Lw, Kw), (1, SEGw)])
                        )
                    else:
                        nc.sync.dma_start(
                            at[k - 1][:, :],
                            _dram_view(a_d, ((k - 1) * BLOC + r0) * TP + 8, [(TP, 2), (Lw, Kw), (1, SEGw)]),
                        )

                b1e, b1o = pair2(b1t[:], 0), pair2(b1t[:], 1)
                b2e, b2o = pair2(b2t[:], 0), pair2(b2t[:], 1)

                # ---- pair condensation (no x dependency) ----
                v.tensor_mul(e10[:], b1o, b2e)
                v.tensor_mul(e11[:], b1o, b1e)
                v.tensor_add(e11[:], e11[:], b2o)

                # ---- FIR (POOL_TAPS taps multiplied+summed on GpSimd) ----
                if POOL_TAPS > 0:
                    pk = pool_taps
                    g.tensor_mul(pp[:], at[pk[0] - 1][:], yt[:, 6 - pk[0] : 6 - pk[0] + SEGw])
                    for k in pk[1:]:
                        g.tensor_mul(ptmp[:], at[k - 1][:], yt[:, 6 - k : 6 - k + SEGw])
                        g.tensor_add(pp[:], pp[:], ptmp[:])
                dk = dve_taps
                v.tensor_mul(xt[:], at[dk[0] - 1][:], yt[:, 6 - dk[0] : 6 - dk[0] + SEGw])
                v.tensor_add(xt[:], xt[:], yt[:, 6 : 6 + SEGw])
                for k in dk[1:]:
                    v.tensor_mul(tmp[:], at[k - 1][:], yt[:, 6 - k : 6 - k + SEGw])
                    v.tensor_add(xt[:], xt[:], tmp[:])
                if POOL_TAPS > 0:
                    v.tensor_add(xt[:], xt[:], pp[:])

                xe, xo = pair2(xt[:], 0), pair2(xt[:], 1)
                v.tensor_mul(f2[:], b1o, xe)
                v.tensor_add(f2[:], f2[:], xo)

                v.memset(s1[:, 0:1], 0.0)
                v.memset(s2[:, 0:1], 0.0)
                v.memset(yo[:, 0:2], 0.0)

                s1d, s1s = s1[:, 1:], s1[:, 0:HPw]
                s2d, s2s = s2[:, 1:], s2[:, 0:HPw]
                yod = yo[:, 2:]
                yo_even = pair2(yod, 0)
                yo_odd = pair2(yod, 1)
                yo_odd_sh = pair2(yo[:, 0:SEGw], 1)

                def tts2(out2, d0, d1):
                    v.tensor_tensor_scan(out2, d0, d1, 0.0, MULT, ADD)

                u2e = g if POOL_U >= 1 else v

                # ---- sweep 0 ----
                tts2(s1d, b2e, xe)
                u2e.tensor_mul(u2[:], e10[:], s1s)
                u2e.tensor_add(u2[:], u2[:], f2[:])
                tts2(s2d, e11[:], u2[:])

                for sw in range(1, NSWEEP):
                    last = sw == NSWEEP - 1
                    v.tensor_mul(u1[:], b1e, s2s)
                    v.tensor_add(u1[:], u1[:], xe)
                    tts2(s1d, b2e, u1[:])
                    u2e.tensor_mul(u2[:], e10[:], s1s)
                    u2e.tensor_add(u2[:], u2[:], f2[:])
                    tts2(yo_odd if last else s2d, e11[:], u2[:])

                v.tensor_mul(u1[:], b1e, yo_odd_sh)
                v.tensor_add(u1[:], u1[:], xe)
                tts2(yo_even, b2e, u1[:])

                nc.sync.dma_start(
                    _dram_view(out_d, r0 * Kw * Lw, [(Kw * Lw, 2), (Lw, Kw), (1, Lw)]),
                    yo[:, 2 + W : 2 + W + Lw],
                )

    nc.compile()
    return nc


def _build_program():
    if WIDE:
        return _build_program_wide()
    nc = bacc.Bacc("TRN2", target_bir_lowering=False, debug=False)

    y_d = nc.dram_tensor("y_pad", (BLOC, TP), mybir.dt.float32, kind="ExternalInput")
    a_d = nc.dram_tensor("a_tap", (6, BLOC, TP), mybir.dt.float32, kind="ExternalInput")
    b1_d = nc.dram_tensor("b1_pad", (BLOC, TP), mybir.dt.float32, kind="ExternalInput")
    b2_d = nc.dram_tensor("b2_pad", (BLOC, TP), mybir.dt.float32, kind="ExternalInput")
    out_d = nc.dram_tensor("y_out", (BLOC, K * L), mybir.dt.float32, kind="ExternalOutput")

    v = nc.vector
    g = nc.gpsimd

    def pair(ap3, sel):
        # [128, NB, 2*n] -> even (sel=0) / odd (sel=1) view [128, NB, n]
        n = ap3.shape[2]
        assert n % 2 == 0
        return ap3.rearrange("p b (m two) -> p b two m", two=2)[:, :, sel, :]

    bufs = int(os.environ.get("KS_BUFS", "2"))
    with tile.TileContext(nc) as tc:
        with tc.tile_pool(name="main", bufs=bufs) as pool:
            for s in range(NSLAB):
                rows = [s * NB + i for i in range(NB)]

                yt = pool.tile([K, NB, 762], mybir.dt.float32, name=f"yt{s}", tag="yt")
                at = [
                    pool.tile([K, NB, SEG], mybir.dt.float32, name=f"at{k}_{s}", tag=f"at{k}")
                    for k in range(1, 7)
                ]
                b1t = pool.tile([K, NB, SEG], mybir.dt.float32, name=f"b1t{s}", tag="b1t")
                b2t = pool.tile([K, NB, SEG], mybir.dt.float32, name=f"b2t{s}", tag="b2t")
                xt = pool.tile([K, NB, SEG], mybir.dt.float32, name=f"xt{s}", tag="xt")
                tmp = pool.tile([K, NB, SEG], mybir.dt.float32, name=f"tmp{s}", tag="tmp")
                if POOL_TAPS > 0:
                    pp = pool.tile([K, NB, SEG], mybir.dt.float32, name=f"pp{s}", tag="pp")
                    ptmp = pool.tile([K, NB, SEG], mybir.dt.float32, name=f"ptmp{s}", tag="ptmp")
                e10 = pool.tile([K, NB, HP], mybir.dt.float32, name=f"e10_{s}", tag="e10")
                e11 = pool.tile([K, NB, HP], mybir.dt.float32, name=f"e11_{s}", tag="e11")
                f2 = pool.tile([K, NB, HP], mybir.dt.float32, name=f"f2_{s}", tag="f2")
                u1 = pool.tile([K, NB, HP], mybir.dt.float32, name=f"u1_{s}", tag="u1")
                u2 = pool.tile([K, NB, HP], mybir.dt.float32, name=f"u2_{s}", tag="u2")
                s1 = pool.tile([K, NB, HP + 1], mybir.dt.float32, name=f"s1_{s}", tag="s1")
                s2 = pool.tile([K, NB, HP + 1], mybir.dt.float32, name=f"s2_{s}", tag="s2")
                yo = pool.tile([K, NB, SEG + 2], mybir.dt.float32, name=f"yo{s}", tag="yo")

                # ---- input DMAs (chunk-strided views of the padded rows) ----
                # Pool's FIR taps (7-POOL_TAPS..6) load first so GpSimd can
                # start before VectorE finishes its own taps.
                _order = os.environ.get("KS_TAP_ORDER", "seq")
                if _order == "pool_first":
                    tap_order = list(range(7 - POOL_TAPS, 7)) + list(range(1, 7 - POOL_TAPS))
                elif _order == "interleave":
                    a_, b_ = list(range(1, 7 - POOL_TAPS)), list(range(7 - POOL_TAPS, 7))
                    tap_order = [x for pair_ in zip(a_, b_) for x in pair_]
                    tap_order += a_[len(b_):] + b_[len(a_):]
                else:
                    tap_order = list(range(1, 7))
                for i, r in enumerate(rows):
                    nc.sync.dma_start(
                        yt[:, i, :], _dram_view(y_d, r * TP + 2, [(L, K), (1, 762)])
                    )
                    for k in tap_order:
                        nc.sync.dma_start(
                            at[k - 1][:, i, :],
                            _dram_view(a_d, ((k - 1) * BLOC + r) * TP + 8, [(L, K), (1, SEG)]),
                        )
                    nc.sync.dma_start(
                        b1t[:, i, :], _dram_view(b1_d, r * TP + 8, [(L, K), (1, SEG)])
                    )
                    nc.sync.dma_start(
                        b2t[:, i, :], _dram_view(b2_d, r * TP + 8, [(L, K), (1, SEG)])
                    )

                xe, xo = pair(xt, 0), pair(xt, 1)
                b1e, b1o = pair(b1t, 0), pair(b1t, 1)
                b2e, b2o = pair(b2t, 0), pair(b2t, 1)

                # ---- pair condensation (E has no x dependency: emit first) ----
                ee = g if POOL_EF >= 1 else v
                ee.tensor_mul(e10[:], b1o, b2e)
                ee.tensor_mul(e11[:], b1o, b1e)
                ee.tensor_add(e11[:], e11[:], b2o)

                # ---- FIR: x[j] = y[j] + sum_k A_k[j] * y[j-k] ----
                # taps 1..6-POOL_TAPS accumulate on VectorE; the top POOL_TAPS
                # taps are multiplied+summed on GpSimd and added in once.
                dve_hi = 6 - POOL_TAPS
                if POOL_TAPS > 0:
                    k0 = dve_hi + 1
                    g.tensor_mul(pp[:], at[k0 - 1][:], yt[:, :, 6 - k0 : 6 - k0 + SEG])
                    for k in range(k0 + 1, 7):
                        g.tensor_mul(ptmp[:], at[k - 1][:], yt[:, :, 6 - k : 6 - k + SEG])
                        g.tensor_add(pp[:], pp[:], ptmp[:])
                v.tensor_mul(xt[:], at[0][:], yt[:, :, 5 : 5 + SEG])
                v.tensor_add(xt[:], xt[:], yt[:, :, 6 : 6 + SEG])
                for k in range(2, dve_hi + 1):
                    v.tensor_mul(tmp[:], at[k - 1][:], yt[:, :, 6 - k : 6 - k + SEG])
                    v.tensor_add(xt[:], xt[:], tmp[:])
                if POOL_TAPS > 0:
                    v.tensor_add(xt[:], xt[:], pp[:])

                # ---- f2 (needs x) ----
                fe = g if POOL_EF == 1 else v
                fe.tensor_mul(f2[:], b1o, xe)
                fe.tensor_add(f2[:], f2[:], xo)

                # guard columns (shift reads at m=0 land here; must be finite)
                v.memset(s1[:, :, 0:1], 0.0)
                v.memset(s2[:, :, 0:1], 0.0)
                v.memset(yo[:, :, 0:2], 0.0)

                s1d, s1s = s1[:, :, 1:], s1[:, :, 0:HP]
                s2d, s2s = s2[:, :, 1:], s2[:, :, 0:HP]
                yod = yo[:, :, 2:]                      # [K, NB, SEG]
                yo_even = pair(yod, 0)                  # write: y at even pairs
                yo_odd = pair(yod, 1)                   # write: y at odd pairs
                yo_odd_sh = pair(yo[:, :, 0:SEG], 1)    # read: odd pairs shifted by 1
                yo_even_sh = pair(yo[:, :, 0:SEG], 0)   # read: even pairs shifted by 1

                def tts(out3, d0_3, d1_3):
                    for i in range(NB):
                        v.tensor_tensor_scan(
                            out3[:, i, :], d0_3[:, i, :], d1_3[:, i, :], 0.0, MULT, ADD
                        )

                u2e = g if POOL_U >= 1 else v
                u1e = g if POOL_U >= 2 else v

                # ---- sweep 0 (s2_prev = 0) ----
                tts(s1d, b2e, xe)
                u2e.tensor_mul(u2[:], e10[:], s1s)
                u2e.tensor_add(u2[:], u2[:], f2[:])
                tts(s2d, e11, u2)

                # ---- sweeps 1..NSWEEP-1 ----
                for sw in range(1, NSWEEP):
                    last = sw == NSWEEP - 1
                    s1_out = yo_even if (last and not FINAL_HALF) else s1d
                    s1_sh = yo_even_sh if (last and not FINAL_HALF) else s1s
                    u1e.tensor_mul(u1[:], b1e, s2s)
                    u1e.tensor_add(u1[:], u1[:], xe)
                    tts(s1_out, b2e, u1)
                    u2e.tensor_mul(u2[:], e10[:], s1_sh)
                    u2e.tensor_add(u2[:], u2[:], f2[:])
                    tts(yo_odd if last else s2d, e11, u2)

                if FINAL_HALF:
                    # ---- final even half-sweep against the settled odd samples ----
                    u1e.tensor_mul(u1[:], b1e, yo_odd_sh)
                    u1e.tensor_add(u1[:], u1[:], xe)
                    tts(yo_even, b2e, u1)

                # ---- output DMA (drop warmup; tail pad cut on host) ----
                for i, r in enumerate(rows):
                    nc.sync.dma_start(
                        _dram_view(out_d, r * K * L, [(L, K), (1, L)]),
                        yo[:, i, 2 + W : 2 + SEG],
                    )

    nc.compile()
    return nc


def _prep_inputs(y, A_exc, A_loop):
    y = np.ascontiguousarray(y, dtype=np.float32)
    A_exc = np.ascontiguousarray(A_exc, dtype=np.float32)
    A_loop = np.ascontiguousarray(A_loop, dtype=np.float32)

    y_pad = np.zeros((B, TP), np.float32)
    y_pad[:, PRE : PRE + T] = y
    a_tap = np.zeros((6, B, TP), np.float32)
    for k in range(6):
        a_tap[k, :, PRE : PRE + T] = A_exc[:, :, k]
    b1_pad = np.zeros((B, TP), np.float32)
    b2_pad = np.zeros((B, TP), np.float32)
    b1_pad[:, PRE : PRE + T] = -A_loop[:, :, 0]
    b2_pad[:, PRE : PRE + T] = -A_loop[:, :, 1]

    in_maps = []
    for c in range(NCORES):
        r0, r1 = c * BLOC, (c + 1) * BLOC
        in_maps.append(
            {
                "y_pad": y_pad[r0:r1],
                "a_tap": np.ascontiguousarray(a_tap[:, r0:r1]),
                "b1_pad": b1_pad[r0:r1],
                "b2_pad": b2_pad[r0:r1],
            }
        )
    return in_maps


def _get_program():
    if "nc" not in _compiled:
        _compiled["nc"] = _build_program()
    return _compiled["nc"]


def run(y, A_exc, A_loop, trace=False, **trace_kwargs):
    """Returns (output, BassKernelResults)."""
    nc = _get_program()
    in_maps = _prep_inputs(y, A_exc, A_loop)
    res = run_bass_kernel_spmd(
        nc, in_maps, list(range(NCORES)), trace=trace, **trace_kwargs
    )
    out = np.empty((B, T), np.float32)
    for c in range(NCORES):
        out[c * BLOC : (c + 1) * BLOC] = res.results[c]["y_out"][:, :T]
    return out, res


def kernel(y, A_exc, A_loop):
    out, _ = run(y, A_exc, A_loop)
    return out



# revision 16
# speedup vs baseline: 2.1981x; 2.1981x over previous
"""Trainium2 Bass kernel for the DiffKS pipeline:
  x = invert_lpc(y, A_exc)         (order-6 time-varying FIR)
  out = sample_wise_lpc(x, A_loop) (order-2 time-varying all-pole IIR)

Sharding: pure data-parallel over batch B=48 -> 6 rows per core x 8 cores.

Design (v2, fp16 deinterleaved planes):
  * All inputs are repacked on the host into fp16 even/odd "planes" over the
    half-rate grid (t = 2j / 2j+1), stored per (row, chunk) with the warmup
    halo duplicated. This halves HBM traffic vs fp32 AND makes every on-chip
    elementwise op a unit-stride 2-byte tensor_tensor (DVE 2x perf mode),
    with one big 3-dim-AP DMA per plane group.
  * Time axis is chunked: 42 chunks x Lw=2100 per row; 3 rows x 42 chunks
    = 126 SBUF partitions per slab, 2 slabs per core. Every chunk re-runs
    the recurrence from W=32 samples early with zero initial state;
    |A_loop|<=0.25 contracts the wrong boundary state by >=2x per 2 samples
    (2^-16 by the chunk's real samples - far below the fp16 noise floor).
  * The order-2 IIR is pair-condensed into two coupled first-order
    recurrences over (even, odd) sample pairs; each half-sweep is an exact
    first-order solve via the hardware tensor_tensor_scan (fp32 internal
    state). Two full Gauss-Seidel sweeps (variant A2) leave total error at
    the fp16 rounding floor (~1.9e-3 rel vs the 2e-2 tolerance).
  * Host precomputes the pair-condensation coefficient planes (e10, e11)
    and the combined f2-FIR coefficient planes c0..c6 (pure functions of
    A_exc/A_loop), so the device never materializes x_odd:
      f2 = b1o*xe + xo
         = yo + c0*ye + c1*yo(-1) + c2*ye(-1) + c3*yo(-2) + c4*ye(-2)
              + c5*yo(-3) + c6*ye(-3)
  * Work split: GpSimd(Pool) computes the c0..c3 part of f2 (8 coarse ops,
    one cross-engine edge per slab); DVE does the xe FIR, the c4..c6 tail,
    the u-combines and the 4 scans.
"""

import numpy as np

import sys

for _p in ("/opt/trn_rl_repo",):
    if _p not in sys.path:
        sys.path.insert(0, _p)

from concourse import bacc, bass, mybir, tile
from concourse.bass_utils import run_bass_kernel_spmd

B, T = 48, 88200
NCORES = 8
BLOC = B // NCORES       # 6 batch rows per core
RS = 3                   # rows per slab
NSLAB = BLOC // RS       # 2 slabs
CH = 42                  # chunks per row; RS*CH = 126 partitions
NPART = RS * CH
Lw = T // CH             # 2100 samples per chunk
W = 32                   # warmup samples per chunk (even)
SEG = W + Lw             # 2132
HP = SEG // 2            # 1066 pairs per chunk-segment
HL = Lw // 2             # 1050 real pairs per chunk
PRE = 40                 # leading zero pad (>= W + 8), even
TPh = (PRE + T) // 2     # 44120 half-grid padded row length
HALO = 4                 # extra leading halo elems on the y planes
YW = HP + HALO           # 1070: y plane tile width
NPL = 17                 # coefficient planes: a1..a3, c0..c3, a4..a6, c4..c6,
                         #                     b1e, b2e, e10, e11

MULT = mybir.AluOpType.mult
ADD = mybir.AluOpType.add
f16 = mybir.dt.float16

import os

# pool ops per slab for the f2 partial (see op-list comment in the kernel)
POOL_OPS = tuple(
    int(x) for x in os.environ.get("KS_POOL_OPS", "4,8").split(",")
)
# GS variant: a2x (3 scans/slab, ~2.1e-3 rel) or axy (2 scans, ~7.4e-3 rel)
VARIANT = os.environ.get("KS_V", "a2x")

_compiled = {}


def _dram_view(handle, offset, dims):
    """Raw strided view of a DRAM tensor: dims = [(stride, count), ...]."""
    return bass.AP(handle, offset, [[s, c] for (s, c) in dims])


def _build_program():
    nc = bacc.Bacc("TRN2", target_bir_lowering=False, debug=False)

    # DRAM inputs, per-chunk fp16 layout:
    #   yg:  (BLOC, CH, 2, YW)    [ye, yo] with HALO leading halo elems
    #   pg:  (BLOC, CH, NPL, HP)  coefficient planes (order above)
    yg = nc.dram_tensor("yg", (BLOC, CH, 2, YW), f16, kind="ExternalInput")
    pg = nc.dram_tensor("pg", (BLOC, CH, NPL, HP), f16, kind="ExternalInput")
    out_d = nc.dram_tensor("y_out", (BLOC, 2, CH * HL), f16, kind="ExternalOutput")

    v = nc.vector
    g = nc.gpsimd

    def in_dma(dst, r0, pl0, npl):
        """Load coefficient planes [pl0, pl0+npl) for rows [r0, r0+RS)."""
        nc.sync.dma_start(
            dst,
            _dram_view(
                pg,
                (r0 * CH * NPL + pl0) * HP,
                [(CH * NPL * HP, RS), (NPL * HP, CH), (1, npl * HP)],
            ),
        )

    with tile.TileContext(nc) as tc:
        with tc.tile_pool(name="main", bufs=2) as pool:
            for s in range(NSLAB):
                r0 = s * RS

                yt = pool.tile([NPART, 2 * YW], f16, name=f"yt{s}", tag="yt")
                a1t = pool.tile([NPART, 3 * HP], f16, name=f"a1t{s}", tag="a1t")
                c1t = pool.tile([NPART, 3 * HP], f16, name=f"c1t{s}", tag="c1t")
                a2t = pool.tile([NPART, 3 * HP], f16, name=f"a2t{s}", tag="a2t")
                c2t = pool.tile([NPART, 4 * HP], f16, name=f"c2t{s}", tag="c2t")
                bt = pool.tile([NPART, 4 * HP], f16, name=f"bt{s}", tag="bt")

                xe = pool.tile([NPART, HP + 1], f16, name=f"xe{s}", tag="xe")
                tv = pool.tile([NPART, HP], f16, name=f"tv{s}", tag="tv")
                qv = pool.tile([NPART, HP], f16, name=f"qv{s}", tag="qv")
                pf = pool.tile([NPART, HP], f16, name=f"pf{s}", tag="pf")
                pt = pool.tile([NPART, HP], f16, name=f"pt{s}", tag="pt")
                f2 = pool.tile([NPART, HP], f16, name=f"f2{s}", tag="f2")
                u1 = pool.tile([NPART, HP], f16, name=f"u1{s}", tag="u1")
                u2 = pool.tile([NPART, HP], f16, name=f"u2{s}", tag="u2")
                u2b = pool.tile([NPART, HP], f16, name=f"u2b{s}", tag="u2b")
                s2 = pool.tile([NPART, HP + 1], f16, name=f"s2_{s}", tag="s2")
                yy = pool.tile([NPART, 2 * (HP + 1)], f16, name=f"yy{s}", tag="yy")

                # ---- input DMAs (order = earliest consumer first; the
                # first few are single-plane so DVE/Pool start ASAP) ----
                def y_dma(pl, n):
                    nc.sync.dma_start(
                        yt[:, pl * YW : (pl + n) * YW],
                        _dram_view(
                            yg,
                            (r0 * CH * 2 + pl) * YW,
                            [(CH * 2 * YW, RS), (2 * YW, CH), (1, n * YW)],
                        ),
                    )

                if s == 0:
                    # slab 0: feed DVE's FIR first, then Pool's planes
                    y_dma(1, 1)                   # yo
                    in_dma(a1t[:, 0:HP], r0, 0, 1)            # a1
                    y_dma(0, 1)                   # ye
                    in_dma(a1t[:, HP : 3 * HP], r0, 1, 2)     # a2 a3
                    in_dma(c1t[:, :], r0, 3, 3)   # c0 c1 c2
                    in_dma(a2t[:, :], r0, 6, 3)   # a4 a5 a6
                    in_dma(c2t[:, :], r0, 9, 4)   # c3 c4 c5 c6
                    in_dma(bt[:, :], r0, 13, 4)   # b1e b2e e10 e11
                else:
                    # later slabs: Pool resumes first (DVE is still busy
                    # with the previous slab's chain), so c-planes lead
                    in_dma(c1t[:, :], r0, 3, 3)   # c0 c1 c2
                    y_dma(0, 2)                   # ye yo
                    in_dma(a1t[:, :], r0, 0, 3)   # a1 a2 a3
                    in_dma(a2t[:, :], r0, 6, 3)   # a4 a5 a6
                    in_dma(c2t[:, :], r0, 9, 4)   # c3 c4 c5 c6
                    in_dma(bt[:, :], r0, 13, 4)   # b1e b2e e10 e11

                # plane views
                def yev(d):  # ye[j - d]
                    return yt[:, HALO - d : HALO - d + HP]

                def yov(d):  # yo[j - d]
                    return yt[:, YW + HALO - d : YW + HALO - d + HP]

                def a1v(k):
                    return a1t[:, k * HP : (k + 1) * HP]

                def a2v(k):
                    return a2t[:, k * HP : (k + 1) * HP]

                def c1v(k):
                    return c1t[:, k * HP : (k + 1) * HP]

                def c2v(k):
                    return c2t[:, k * HP : (k + 1) * HP]

                b1e = bt[:, 0:HP]
                b2e = bt[:, HP : 2 * HP]
                e10 = bt[:, 2 * HP : 3 * HP]
                e11 = bt[:, 3 * HP : 4 * HP]

                # xe has a 1-elem zero guard so sh(xe) reads are in-tile
                xeg = xe[:, 0:HP]     # xe[m-1] (shifted) view
                xeb = xe[:, 1 : HP + 1]  # xe[m] view

                # ---- scan/shift guards ----
                v.memset(xe[:, 0:1], 0.0)
                v.memset(s2[:, 0:1], 0.0)
                if VARIANT == "a2x":
                    v.memset(yy[:, 0:1], 0.0)          # yed[0] (sh read)
                else:
                    v.memset(yy[:, HP + 1 : HP + 2], 0.0)  # yod[0] (sh read)

                # ---- f2 tap list: f2 = yo + sum_k c_k * y_shift_k ----
                # taps c0..c2 come from c1t, c3..c6 from c2t.
                f2taps = [
                    (c1v(0), yev(0)),
                    (c1v(1), yov(1)),
                    (c1v(2), yev(1)),
                    (c2v(0), yov(2)),
                    (c2v(1), yev(2)),
                    (c2v(2), yov(3)),
                    (c2v(3), yev(3)),
                ]
                # Pool executes the first POOL_OPS[s] ops of the flat op list
                # [mul0, add_base, mul1, add1, mul2, add2, mul3, add3]:
                # an odd count means the last pool op is a mul whose product
                # (in pt) is folded in by DVE.
                P = POOL_OPS[s]
                ntap_pool = (P + 1) // 2  # taps pool multiplies
                pool_tail_mul = P % 2 == 1
                g.tensor_mul(pf[:], *f2taps[0])
                g.tensor_add(pf[:], pf[:], yov(0))
                for k in range(1, ntap_pool):
                    g.tensor_mul(pt[:], *f2taps[k])
                    if 2 * (k + 1) <= P:
                        g.tensor_add(pf[:], pf[:], pt[:])

                # ---- DVE: xe FIR ----
                v.tensor_mul(xeb, a1v(0), yov(1))
                v.tensor_add(xeb, xeb, yev(0))
                v.tensor_mul(tv[:], a1v(1), yev(1))
                v.tensor_add(xeb, xeb, tv[:])
                v.tensor_mul(tv[:], a1v(2), yov(2))
                v.tensor_add(xeb, xeb, tv[:])
                v.tensor_mul(tv[:], a2v(0), yev(2))
                v.tensor_add(xeb, xeb, tv[:])
                v.tensor_mul(tv[:], a2v(1), yov(3))
                v.tensor_add(xeb, xeb, tv[:])
                v.tensor_mul(tv[:], a2v(2), yev(3))
                v.tensor_add(xeb, xeb, tv[:])

                # ---- DVE: f2 tail (remaining taps) + merge with Pool ----
                v.tensor_mul(qv[:], *f2taps[ntap_pool])
                if pool_tail_mul:
                    v.tensor_add(qv[:], qv[:], pt[:])
                for k in range(ntap_pool + 1, 7):
                    v.tensor_mul(tv[:], *f2taps[k])
                    v.tensor_add(qv[:], qv[:], tv[:])
                v.tensor_add(f2[:], qv[:], pf[:])

                yed = yy[:, 0 : HP + 1]
                yod = yy[:, HP + 1 : 2 * (HP + 1)]

                def tts(out2, d0, d1):
                    v.tensor_tensor_scan(out2, d0, d1, 0.0, MULT, ADD)

                def out_dma(plane, p):
                    nc.sync.dma_start(
                        _dram_view(
                            out_d,
                            (r0 * 2 + p) * CH * HL,
                            [(2 * CH * HL, RS), (HL, CH), (1, HL)],
                        ),
                        plane[:, 1 + W // 2 : 1 + W // 2 + HL],
                    )

                if VARIANT == "a2x":
                    # ---- GS A2x: s1^0 := xe, then a full sweep ----
                    v.tensor_mul(u2[:], e10, xeg)
                    v.tensor_add(u2[:], u2[:], f2[:])
                    tts(s2[:, 1:], e11, u2[:])
                    v.tensor_mul(u1[:], b1e, s2[:, 0:HP])
                    v.tensor_add(u1[:], u1[:], xeb)
                    tts(yed[:, 1:], b2e, u1[:])
                    v.tensor_mul(u2b[:], e10, yed[:, 0:HP])
                    v.tensor_add(u2b[:], u2b[:], f2[:])
                    out_dma(yed, 0)
                    tts(yod[:, 1:], e11, u2b[:])
                    out_dma(yod, 1)
                else:
                    # ---- GS Axy: s1^0 := xe + b2e*sh(xe), odd, even ----
                    # (s2 tile reused for the Neumann start s1h)
                    v.tensor_mul(tv[:], b2e, xeg)
                    v.tensor_add(s2[:, 1:], tv[:], xeb)
                    v.tensor_mul(u2[:], e10, s2[:, 0:HP])
                    v.tensor_add(u2[:], u2[:], f2[:])
                    tts(yod[:, 1:], e11, u2[:])
                    v.tensor_mul(u1[:], b1e, yod[:, 0:HP])
                    v.tensor_add(u1[:], u1[:], xeb)
                    out_dma(yod, 1)
                    tts(yed[:, 1:], b2e, u1[:])
                    out_dma(yed, 0)

    nc.compile()
    return nc


def _prep_inputs(y, A_exc, A_loop):
    y = np.asarray(y, dtype=np.float32)
    A_exc = np.asarray(A_exc, dtype=np.float32)
    A_loop = np.asarray(A_loop, dtype=np.float32)

    TP = PRE + T
    y_pad = np.zeros((B, TP), np.float32)
    y_pad[:, PRE:] = y
    b1 = np.zeros((B, TP), np.float32)
    b2 = np.zeros((B, TP), np.float32)
    b1[:, PRE:] = -A_loop[:, :, 0]
    b2[:, PRE:] = -A_loop[:, :, 1]
    a_pad = np.zeros((B, TP, 6), np.float32)
    a_pad[:, PRE:, :] = A_exc

    # half-grid planes (length TPh)
    ye = y_pad[:, 0::2]
    yo = y_pad[:, 1::2]
    ae = [np.ascontiguousarray(a_pad[:, 0::2, k]) for k in range(6)]
    ao = [np.ascontiguousarray(a_pad[:, 1::2, k]) for k in range(6)]
    b1e, b1o = b1[:, 0::2], b1[:, 1::2]
    b2e, b2o = b2[:, 0::2], b2[:, 1::2]

    e10 = b1o * b2e
    e11 = b1o * b1e + b2o
    c = [
        b1o + ao[0],
        b1o * ae[0] + ao[1],
        b1o * ae[1] + ao[2],
        b1o * ae[2] + ao[3],
        b1o * ae[3] + ao[4],
        b1o * ae[4] + ao[5],
        b1o * ae[5],
    ]

    # full-row plane stacks, fp16
    # order: a1 a2 a3 | c0 c1 c2 | a4 a5 a6 | c3 c4 c5 c6 | b1e b2e e10 e11
    pl = np.stack(
        [ae[0], ae[1], ae[2], c[0], c[1], c[2], ae[3], ae[4], ae[5],
         c[3], c[4], c[5], c[6], b1e, b2e, e10, e11],
        axis=1,
    ).astype(np.float16)          # (B, NPL, TPh)
    yl = np.stack([ye, yo], axis=1).astype(np.float16)  # (B, 2, TPh)

    # per-chunk gather (duplicates the warmup halo; chunk c starts at
    # half-grid index HALO + c*HL for coefficients, c*HL for y-with-halo)
    st_p = pl.strides
    pg = np.lib.stride_tricks.as_strided(
        pl[:, :, HALO:],
        shape=(B, CH, NPL, HP),
        strides=(st_p[0], HL * st_p[2], st_p[1], st_p[2]),
    )
    st_y = yl.strides
    ygs = np.lib.stride_tricks.as_strided(
        yl,
        shape=(B, CH, 2, YW),
        strides=(st_y[0], HL * st_y[2], st_y[1], st_y[2]),
    )

    in_maps = []
    for cix in range(NCORES):
        r0, r1 = cix * BLOC, (cix + 1) * BLOC
        in_maps.append(
            {
                "yg": np.ascontiguousarray(ygs[r0:r1]),
                "pg": np.ascontiguousarray(pg[r0:r1]),
            }
        )
    return in_maps


def _get_program():
    if "nc" not in _compiled:
        _compiled["nc"] = _build_program()
    return _compiled["nc"]


def run(y, A_exc, A_loop, trace=False, **trace_kwargs):
    """Returns (output, BassKernelResults)."""
    nc = _get_program()
    in_maps = _prep_inputs(y, A_exc, A_loop)
    res = run_bass_kernel_spmd(
        nc, in_maps, list(range(NCORES)), trace=trace, **trace_kwargs
    )
    out = np.empty((B, T), np.float32)
    for cix in range(NCORES):
        o = res.results[cix]["y_out"].astype(np.float32)  # (BLOC, 2, CH*HL)
        blk = out[cix * BLOC : (cix + 1) * BLOC]
        blk[:, 0::2] = o[:, 0, :]
        blk[:, 1::2] = o[:, 1, :]
    return out, res


def kernel(y, A_exc, A_loop):
    out, _ = run(y, A_exc, A_loop)
    return out


# revision 22
# speedup vs baseline: 2.2293x; 1.0142x over previous
"""Trainium2 Bass kernel for the DiffKS pipeline:
  x = invert_lpc(y, A_exc)         (order-6 time-varying FIR)
  out = sample_wise_lpc(x, A_loop) (order-2 time-varying all-pole IIR)

Sharding: pure data-parallel over batch B=48 -> 6 rows per core x 8 cores.

Design (v2, fp16 deinterleaved planes):
  * All inputs are repacked on the host into fp16 even/odd "planes" over the
    half-rate grid (t = 2j / 2j+1), stored per (row, chunk) with the warmup
    halo duplicated. This halves HBM traffic vs fp32 AND makes every on-chip
    elementwise op a unit-stride 2-byte tensor_tensor (DVE 2x perf mode),
    with one big 3-dim-AP DMA per plane group.
  * Time axis is chunked: 42 chunks x Lw=2100 per row; 3 rows x 42 chunks
    = 126 SBUF partitions per slab, 2 slabs per core. Every chunk re-runs
    the recurrence from W=32 samples early with zero initial state;
    |A_loop|<=0.25 contracts the wrong boundary state by >=2x per 2 samples
    (2^-16 by the chunk's real samples - far below the fp16 noise floor).
  * The order-2 IIR is pair-condensed into two coupled first-order
    recurrences over (even, odd) sample pairs; each half-sweep is an exact
    first-order solve via the hardware tensor_tensor_scan (fp32 internal
    state). Two full Gauss-Seidel sweeps (variant A2) leave total error at
    the fp16 rounding floor (~1.9e-3 rel vs the 2e-2 tolerance).
  * Host precomputes the pair-condensation coefficient planes (e10, e11)
    and the combined f2-FIR coefficient planes c0..c6 (pure functions of
    A_exc/A_loop), so the device never materializes x_odd:
      f2 = b1o*xe + xo
         = yo + c0*ye + c1*yo(-1) + c2*ye(-1) + c3*yo(-2) + c4*ye(-2)
              + c5*yo(-3) + c6*ye(-3)
  * Work split: GpSimd(Pool) computes the c0..c3 part of f2 (8 coarse ops,
    one cross-engine edge per slab); DVE does the xe FIR, the c4..c6 tail,
    the u-combines and the 4 scans.
"""

import numpy as np

import sys

for _p in ("/opt/trn_rl_repo",):
    if _p not in sys.path:
        sys.path.insert(0, _p)

from concourse import bacc, bass, mybir, tile
from concourse.bass_utils import run_bass_kernel_spmd

B, T = 48, 88200
NCORES = 8
BLOC = B // NCORES       # 6 batch rows per core
RS = 3                   # rows per slab
NSLAB = BLOC // RS       # 2 slabs
CH = 42                  # chunks per row; RS*CH = 126 partitions
NPART = RS * CH
Lw = T // CH             # 2100 samples per chunk
W = 32                   # warmup samples per chunk (even)
SEG = W + Lw             # 2132
HP = SEG // 2            # 1066 pairs per chunk-segment
HL = Lw // 2             # 1050 real pairs per chunk
PRE = 40                 # leading zero pad (>= W + 8), even
TPh = (PRE + T) // 2     # 44120 half-grid padded row length
HALO = 4                 # extra leading halo elems on the y planes
YW = HP + HALO           # 1070: y plane tile width
NPL = 17                 # coefficient planes: a1..a3, c0..c3, a4..a6, c4..c6,
                         #                     b1e, b2e, e10, e11

MULT = mybir.AluOpType.mult
ADD = mybir.AluOpType.add
f16 = mybir.dt.float16

import os

# pool ops per slab for the f2 partial (see op-list comment in the kernel)
POOL_OPS = tuple(
    int(x) for x in os.environ.get("KS_POOL_OPS", "4,8").split(",")
)
# GS variant: a2x (3 scans/slab, ~2.1e-3 rel) or axy (2 scans, ~7.4e-3 rel)
VARIANT = os.environ.get("KS_V", "a2x")
ORDER0 = os.environ.get("KS_ORDER0", "ac")

_compiled = {}


def _dram_view(handle, offset, dims):
    """Raw strided view of a DRAM tensor: dims = [(stride, count), ...]."""
    return bass.AP(handle, offset, [[s, c] for (s, c) in dims])


def _build_program():
    nc = bacc.Bacc("TRN2", target_bir_lowering=False, debug=False)

    # DRAM inputs, per-chunk fp16 layout:
    #   yg:  (BLOC, CH, 2, YW)    [ye, yo] with HALO leading halo elems
    #   pg:  (BLOC, CH, NPL, HP)  coefficient planes (order above)
    yg = nc.dram_tensor("yg", (BLOC, CH, 2, YW), f16, kind="ExternalInput")
    pg = nc.dram_tensor("pg", (BLOC, CH, NPL, HP), f16, kind="ExternalInput")
    out_d = nc.dram_tensor("y_out", (BLOC, 2, CH * HL), f16, kind="ExternalOutput")

    v = nc.vector
    g = nc.gpsimd

    def in_dma(dst, r0, pl0, npl):
        """Load coefficient planes [pl0, pl0+npl) for rows [r0, r0+RS)."""
        nc.sync.dma_start(
            dst,
            _dram_view(
                pg,
                (r0 * CH * NPL + pl0) * HP,
                [(CH * NPL * HP, RS), (NPL * HP, CH), (1, npl * HP)],
            ),
        )

    with tile.TileContext(nc) as tc:
        with tc.tile_pool(name="main", bufs=int(os.environ.get("KS_BUFS", "2"))) as pool:
            for s in range(NSLAB):
                r0 = s * RS

                yt = pool.tile([NPART, 2 * YW], f16, name=f"yt{s}", tag="yt")
                a1t = pool.tile([NPART, 3 * HP], f16, name=f"a1t{s}", tag="a1t")
                c1t = pool.tile([NPART, 3 * HP], f16, name=f"c1t{s}", tag="c1t")
                a2t = pool.tile([NPART, 3 * HP], f16, name=f"a2t{s}", tag="a2t")
                c2t = pool.tile([NPART, 4 * HP], f16, name=f"c2t{s}", tag="c2t")
                bt = pool.tile([NPART, 4 * HP], f16, name=f"bt{s}", tag="bt")

                xe = pool.tile([NPART, HP + 1], f16, name=f"xe{s}", tag="xe")
                tv = pool.tile([NPART, HP], f16, name=f"tv{s}", tag="tv")
                qv = pool.tile([NPART, HP], f16, name=f"qv{s}", tag="qv")
                pf = pool.tile([NPART, HP], f16, name=f"pf{s}", tag="pf")
                pt = pool.tile([NPART, HP], f16, name=f"pt{s}", tag="pt")
                f2 = pool.tile([NPART, HP], f16, name=f"f2{s}", tag="f2")
                u1 = pool.tile([NPART, HP], f16, name=f"u1{s}", tag="u1")
                u2 = pool.tile([NPART, HP], f16, name=f"u2{s}", tag="u2")
                u2b = pool.tile([NPART, HP], f16, name=f"u2b{s}", tag="u2b")
                s2 = pool.tile([NPART, HP + 1], f16, name=f"s2_{s}", tag="s2")
                yy = pool.tile([NPART, 2 * (HP + 1)], f16, name=f"yy{s}", tag="yy")

                # ---- input DMAs (order = earliest consumer first; the
                # first few are single-plane so DVE/Pool start ASAP) ----
                def y_dma(pl, n):
                    nc.sync.dma_start(
                        yt[:, pl * YW : (pl + n) * YW],
                        _dram_view(
                            yg,
                            (r0 * CH * 2 + pl) * YW,
                            [(CH * 2 * YW, RS), (2 * YW, CH), (1, n * YW)],
                        ),
                    )

                if s == 0:
                    # slab 0: feed DVE's FIR first, then Pool's planes
                    y_dma(1, 1)                   # yo
                    in_dma(a1t[:, 0:HP], r0, 0, 1)            # a1
                    y_dma(0, 1)                   # ye
                    in_dma(a1t[:, HP : 3 * HP], r0, 1, 2)     # a2 a3
                    if ORDER0 == "ac":
                        in_dma(a2t[:, :], r0, 6, 3)   # a4 a5 a6
                        in_dma(c1t[:, :], r0, 3, 3)   # c0 c1 c2
                    else:
                        in_dma(c1t[:, :], r0, 3, 3)   # c0 c1 c2
                        in_dma(a2t[:, :], r0, 6, 3)   # a4 a5 a6
                    in_dma(c2t[:, :], r0, 9, 4)   # c3 c4 c5 c6
                    in_dma(bt[:, :], r0, 13, 4)   # b1e b2e e10 e11
                else:
                    # later slabs: Pool resumes first (DVE is still busy
                    # with the previous slab's chain), so c-planes lead
                    in_dma(c1t[:, :], r0, 3, 3)   # c0 c1 c2
                    y_dma(0, 2)                   # ye yo
                    in_dma(a1t[:, :], r0, 0, 3)   # a1 a2 a3
                    in_dma(a2t[:, :], r0, 6, 3)   # a4 a5 a6
                    in_dma(c2t[:, :], r0, 9, 4)   # c3 c4 c5 c6
                    in_dma(bt[:, :], r0, 13, 4)   # b1e b2e e10 e11

                # plane views
                def yev(d):  # ye[j - d]
                    return yt[:, HALO - d : HALO - d + HP]

                def yov(d):  # yo[j - d]
                    return yt[:, YW + HALO - d : YW + HALO - d + HP]

                def a1v(k):
                    return a1t[:, k * HP : (k + 1) * HP]

                def a2v(k):
                    return a2t[:, k * HP : (k + 1) * HP]

                def c1v(k):
                    return c1t[:, k * HP : (k + 1) * HP]

                def c2v(k):
                    return c2t[:, k * HP : (k + 1) * HP]

                b1e = bt[:, 0:HP]
                b2e = bt[:, HP : 2 * HP]
                e10 = bt[:, 2 * HP : 3 * HP]
                e11 = bt[:, 3 * HP : 4 * HP]

                # xe has a 1-elem zero guard so sh(xe) reads are in-tile
                xeg = xe[:, 0:HP]     # xe[m-1] (shifted) view
                xeb = xe[:, 1 : HP + 1]  # xe[m] view

                # ---- scan/shift guards ----
                v.memset(xe[:, 0:1], 0.0)
                v.memset(s2[:, 0:1], 0.0)
                if VARIANT == "a2x":
                    v.memset(yy[:, 0:1], 0.0)          # yed[0] (sh read)
                else:
                    v.memset(yy[:, HP + 1 : HP + 2], 0.0)  # yod[0] (sh read)

                # ---- f2 tap list: f2 = yo + sum_k c_k * y_shift_k ----
                # taps c0..c2 come from c1t, c3..c6 from c2t.
                f2taps = [
                    (c1v(0), yev(0)),
                    (c1v(1), yov(1)),
                    (c1v(2), yev(1)),
                    (c2v(0), yov(2)),
                    (c2v(1), yev(2)),
                    (c2v(2), yov(3)),
                    (c2v(3), yev(3)),
                ]
                # Pool executes the first POOL_OPS[s] ops of the flat op list
                # [mul0, add_base, mul1, add1, mul2, add2, mul3, add3]:
                # an odd count means the last pool op is a mul whose product
                # (in pt) is folded in by DVE.
                P = POOL_OPS[s]
                ntap_pool = (P + 1) // 2  # taps pool multiplies
                pool_tail_mul = P % 2 == 1
                g.tensor_mul(pf[:], *f2taps[0])
                g.tensor_add(pf[:], pf[:], yov(0))
                for k in range(1, ntap_pool):
                    g.tensor_mul(pt[:], *f2taps[k])
                    if 2 * (k + 1) <= P:
                        g.tensor_add(pf[:], pf[:], pt[:])

                # ---- DVE: xe FIR ----
                v.tensor_mul(xeb, a1v(0), yov(1))
                v.tensor_add(xeb, xeb, yev(0))
                v.tensor_mul(tv[:], a1v(1), yev(1))
                v.tensor_add(xeb, xeb, tv[:])
                v.tensor_mul(tv[:], a1v(2), yov(2))
                v.tensor_add(xeb, xeb, tv[:])
                v.tensor_mul(tv[:], a2v(0), yev(2))
                v.tensor_add(xeb, xeb, tv[:])
                v.tensor_mul(tv[:], a2v(1), yov(3))
                v.tensor_add(xeb, xeb, tv[:])
                v.tensor_mul(tv[:], a2v(2), yev(3))
                v.tensor_add(xeb, xeb, tv[:])

                # ---- DVE: f2 tail (remaining taps) + merge with Pool ----
                v.tensor_mul(qv[:], *f2taps[ntap_pool])
                if pool_tail_mul:
                    v.tensor_add(qv[:], qv[:], pt[:])
                for k in range(ntap_pool + 1, 7):
                    v.tensor_mul(tv[:], *f2taps[k])
                    v.tensor_add(qv[:], qv[:], tv[:])
                v.tensor_add(f2[:], qv[:], pf[:])

                yed = yy[:, 0 : HP + 1]
                yod = yy[:, HP + 1 : 2 * (HP + 1)]

                def tts(out2, d0, d1):
                    v.tensor_tensor_scan(out2, d0, d1, 0.0, MULT, ADD)

                def out_dma(plane, p):
                    nc.sync.dma_start(
                        _dram_view(
                            out_d,
                            (r0 * 2 + p) * CH * HL,
                            [(2 * CH * HL, RS), (HL, CH), (1, HL)],
                        ),
                        plane[:, 1 + W // 2 : 1 + W // 2 + HL],
                    )

                if VARIANT == "a2x":
                    # ---- GS A2x: s1^0 := xe, then a full sweep ----
                    v.tensor_mul(u2[:], e10, xeg)
                    v.tensor_add(u2[:], u2[:], f2[:])
                    tts(s2[:, 1:], e11, u2[:])
                    v.tensor_mul(u1[:], b1e, s2[:, 0:HP])
                    v.tensor_add(u1[:], u1[:], xeb)
                    tts(yed[:, 1:], b2e, u1[:])
                    v.tensor_mul(u2b[:], e10, yed[:, 0:HP])
                    v.tensor_add(u2b[:], u2b[:], f2[:])
                    out_dma(yed, 0)
                    tts(yod[:, 1:], e11, u2b[:])
                    out_dma(yod, 1)
                else:
                    # ---- GS Axy: s1^0 := xe + b2e*sh(xe), odd, even ----
                    # (s2 tile reused for the Neumann start s1h)
                    v.tensor_mul(tv[:], b2e, xeg)
                    v.tensor_add(s2[:, 1:], tv[:], xeb)
                    v.tensor_mul(u2[:], e10, s2[:, 0:HP])
                    v.tensor_add(u2[:], u2[:], f2[:])
                    tts(yod[:, 1:], e11, u2[:])
                    v.tensor_mul(u1[:], b1e, yod[:, 0:HP])
                    v.tensor_add(u1[:], u1[:], xeb)
                    out_dma(yod, 1)
                    tts(yed[:, 1:], b2e, u1[:])
                    out_dma(yed, 0)

    nc.compile()
    return nc


def _prep_inputs(y, A_exc, A_loop):
    y = np.asarray(y, dtype=np.float32)
    A_exc = np.asarray(A_exc, dtype=np.float32)
    A_loop = np.asarray(A_loop, dtype=np.float32)

    TP = PRE + T
    y_pad = np.zeros((B, TP), np.float32)
    y_pad[:, PRE:] = y
    b1 = np.zeros((B, TP), np.float32)
    b2 = np.zeros((B, TP), np.float32)
    b1[:, PRE:] = -A_loop[:, :, 0]
    b2[:, PRE:] = -A_loop[:, :, 1]
    a_pad = np.zeros((B, TP, 6), np.float32)
    a_pad[:, PRE:, :] = A_exc

    # half-grid planes (length TPh)
    ye = y_pad[:, 0::2]
    yo = y_pad[:, 1::2]
    ae = [np.ascontiguousarray(a_pad[:, 0::2, k]) for k in range(6)]
    ao = [np.ascontiguousarray(a_pad[:, 1::2, k]) for k in range(6)]
    b1e, b1o = b1[:, 0::2], b1[:, 1::2]
    b2e, b2o = b2[:, 0::2], b2[:, 1::2]

    e10 = b1o * b2e
    e11 = b1o * b1e + b2o
    c = [
        b1o + ao[0],
        b1o * ae[0] + ao[1],
        b1o * ae[1] + ao[2],
        b1o * ae[2] + ao[3],
        b1o * ae[3] + ao[4],
        b1o * ae[4] + ao[5],
        b1o * ae[5],
    ]

    # full-row plane stacks, fp16
    # order: a1 a2 a3 | c0 c1 c2 | a4 a5 a6 | c3 c4 c5 c6 | b1e b2e e10 e11
    pl = np.stack(
        [ae[0], ae[1], ae[2], c[0], c[1], c[2], ae[3], ae[4], ae[5],
         c[3], c[4], c[5], c[6], b1e, b2e, e10, e11],
        axis=1,
    ).astype(np.float16)          # (B, NPL, TPh)
    yl = np.stack([ye, yo], axis=1).astype(np.float16)  # (B, 2, TPh)

    # per-chunk gather (duplicates the warmup halo; chunk c starts at
    # half-grid index HALO + c*HL for coefficients, c*HL for y-with-halo)
    st_p = pl.strides
    pg = np.lib.stride_tricks.as_strided(
        pl[:, :, HALO:],
        shape=(B, CH, NPL, HP),
        strides=(st_p[0], HL * st_p[2], st_p[1], st_p[2]),
    )
    st_y = yl.strides
    ygs = np.lib.stride_tricks.as_strided(
        yl,
        shape=(B, CH, 2, YW),
        strides=(st_y[0], HL * st_y[2], st_y[1], st_y[2]),
    )

    in_maps = []
    for cix in range(NCORES):
        r0, r1 = cix * BLOC, (cix + 1) * BLOC
        in_maps.append(
            {
                "yg": np.ascontiguousarray(ygs[r0:r1]),
                "pg": np.ascontiguousarray(pg[r0:r1]),
            }
        )
    return in_maps


def _get_program():
    if "nc" not in _compiled:
        _compiled["nc"] = _build_program()
    return _compiled["nc"]


def run(y, A_exc, A_loop, trace=False, **trace_kwargs):
    """Returns (output, BassKernelResults)."""
    nc = _get_program()
    in_maps = _prep_inputs(y, A_exc, A_loop)
    res = run_bass_kernel_spmd(
        nc, in_maps, list(range(NCORES)), trace=trace, **trace_kwargs
    )
    out = np.empty((B, T), np.float32)
    for cix in range(NCORES):
        o = res.results[cix]["y_out"].astype(np.float32)  # (BLOC, 2, CH*HL)
        blk = out[cix * BLOC : (cix + 1) * BLOC]
        blk[:, 0::2] = o[:, 0, :]
        blk[:, 1::2] = o[:, 1, :]
    return out, res


def kernel(y, A_exc, A_loop):
    out, _ = run(y, A_exc, A_loop)
    return out


# revision 24
# speedup vs baseline: 2.2360x; 1.0030x over previous
"""Trainium2 Bass kernel for the DiffKS pipeline:
  x = invert_lpc(y, A_exc)         (order-6 time-varying FIR)
  out = sample_wise_lpc(x, A_loop) (order-2 time-varying all-pole IIR)

Sharding: pure data-parallel over batch B=48 -> 6 rows per core x 8 cores.

Design (fp16 deinterleaved planes; cost-model timeline 51.8us vs the
115.9us fp32 baseline):
  * All inputs are repacked on the host into fp16 even/odd "planes" over the
    half-rate grid (t = 2j / 2j+1), stored per (row, chunk) with the warmup
    halo duplicated. This halves HBM traffic vs fp32 AND makes every on-chip
    elementwise op a unit-stride 2-byte tensor_tensor (DVE 2x perf mode),
    loaded with a few big 3-dim-AP DMAs per slab, ordered so the engines
    start as early as possible.
  * Time axis is chunked: 42 chunks x Lw=2100 per row; 3 rows x 42 chunks
    = 126 SBUF partitions per slab, 2 slabs per core. Every chunk re-runs
    the recurrence from W=32 samples early with zero initial state;
    |A_loop|<=0.25 contracts the wrong boundary state by >=2x per 2 samples
    (2^-16 by the chunk's real samples - far below the fp16 noise floor).
  * The order-2 IIR is pair-condensed into two coupled first-order
    recurrences over (even, odd) sample pairs; each half-sweep is an exact
    first-order solve via the hardware tensor_tensor_scan (fp32 internal
    state). Gauss-Seidel variant "A2x": sweep 0 approximates the even chain
    by s1 := xe (the even-even coupling b2e only enters through the tiny
    e10 = b1o*b2e product, so the first scan can be skipped), then one full
    sweep; measured 2.1e-3 rel error vs the 2e-2 tolerance (fp16 floor is
    1.9e-3). The final scan of the last slab is split in half (chained via
    an initial=AP scan) so its output DMA overlaps the second half.
  * Host precomputes the pair-condensation coefficient planes (e10, e11)
    and the combined f2-FIR coefficient planes c0..c6 (pure functions of
    A_exc/A_loop), so the device never materializes x_odd:
      f2 = b1o*xe + xo
         = yo + c0*ye + c1*yo(-1) + c2*ye(-1) + c3*yo(-2) + c4*ye(-2)
              + c5*yo(-3) + c6*ye(-3)
  * Work split (engine-balanced, measured on the instruction-cost timeline):
    GpSimd(Pool) computes the first POOL_OPS[s] ops of the f2 tap list
    (4 ops slab 0 - it must finish before DVE's merge - and 8 ops slab 1,
    which has the whole previous chain window); DVE does the xe FIR, the
    f2 tail, the u-combines and the 3 scans per slab. Per-slab DMA order
    differs: slab 0 feeds DVE's FIR taps first, later slabs feed Pool's
    c-planes first since DVE is still draining the previous chain.
"""

import numpy as np

import sys

for _p in ("/opt/trn_rl_repo",):
    if _p not in sys.path:
        sys.path.insert(0, _p)

from concourse import bacc, bass, mybir, tile
from concourse.bass_utils import run_bass_kernel_spmd

B, T = 48, 88200
NCORES = 8
BLOC = B // NCORES       # 6 batch rows per core
RS = 3                   # rows per slab
NSLAB = BLOC // RS       # 2 slabs
CH = 42                  # chunks per row; RS*CH = 126 partitions
NPART = RS * CH
Lw = T // CH             # 2100 samples per chunk
W = 32                   # warmup samples per chunk (even)
SEG = W + Lw             # 2132
HP = SEG // 2            # 1066 pairs per chunk-segment
HL = Lw // 2             # 1050 real pairs per chunk
PRE = 40                 # leading zero pad (>= W + 8), even
TPh = (PRE + T) // 2     # 44120 half-grid padded row length
HALO = 4                 # extra leading halo elems on the y planes
YW = HP + HALO           # 1070: y plane tile width
NPL = 17                 # coefficient planes: a1..a3, c0..c3, a4..a6, c4..c6,
                         #                     b1e, b2e, e10, e11

MULT = mybir.AluOpType.mult
ADD = mybir.AluOpType.add
f16 = mybir.dt.float16

import os

# pool ops per slab for the f2 partial (see op-list comment in the kernel)
POOL_OPS = tuple(
    int(x) for x in os.environ.get("KS_POOL_OPS", "4,8").split(",")
)
# GS variant: a2x (3 scans/slab, ~2.1e-3 rel) or axy (2 scans, ~7.4e-3 rel)
VARIANT = os.environ.get("KS_V", "a2x")
ORDER0 = os.environ.get("KS_ORDER0", "ac")

_compiled = {}


def _dram_view(handle, offset, dims):
    """Raw strided view of a DRAM tensor: dims = [(stride, count), ...]."""
    return bass.AP(handle, offset, [[s, c] for (s, c) in dims])


def _build_program():
    nc = bacc.Bacc("TRN2", target_bir_lowering=False, debug=False)

    # DRAM inputs, per-chunk fp16 layout:
    #   yg:  (BLOC, CH, 2, YW)    [ye, yo] with HALO leading halo elems
    #   pg:  (BLOC, CH, NPL, HP)  coefficient planes (order above)
    yg = nc.dram_tensor("yg", (BLOC, CH, 2, YW), f16, kind="ExternalInput")
    pg = nc.dram_tensor("pg", (BLOC, CH, NPL, HP), f16, kind="ExternalInput")
    out_d = nc.dram_tensor("y_out", (BLOC, 2, CH * HL), f16, kind="ExternalOutput")

    v = nc.vector
    g = nc.gpsimd

    def in_dma(dst, r0, pl0, npl):
        """Load coefficient planes [pl0, pl0+npl) for rows [r0, r0+RS)."""
        nc.sync.dma_start(
            dst,
            _dram_view(
                pg,
                (r0 * CH * NPL + pl0) * HP,
                [(CH * NPL * HP, RS), (NPL * HP, CH), (1, npl * HP)],
            ),
        )

    with tile.TileContext(nc) as tc:
        with tc.tile_pool(name="main", bufs=int(os.environ.get("KS_BUFS", "2"))) as pool:
            for s in range(NSLAB):
                r0 = s * RS

                yt = pool.tile([NPART, 2 * YW], f16, name=f"yt{s}", tag="yt")
                a1t = pool.tile([NPART, 3 * HP], f16, name=f"a1t{s}", tag="a1t")
                c1t = pool.tile([NPART, 3 * HP], f16, name=f"c1t{s}", tag="c1t")
                a2t = pool.tile([NPART, 3 * HP], f16, name=f"a2t{s}", tag="a2t")
                c2t = pool.tile([NPART, 4 * HP], f16, name=f"c2t{s}", tag="c2t")
                bt = pool.tile([NPART, 4 * HP], f16, name=f"bt{s}", tag="bt")

                xe = pool.tile([NPART, HP + 1], f16, name=f"xe{s}", tag="xe")
                tv = pool.tile([NPART, HP], f16, name=f"tv{s}", tag="tv")
                qv = pool.tile([NPART, HP], f16, name=f"qv{s}", tag="qv")
                pf = pool.tile([NPART, HP], f16, name=f"pf{s}", tag="pf")
                pt = pool.tile([NPART, HP], f16, name=f"pt{s}", tag="pt")
                f2 = pool.tile([NPART, HP], f16, name=f"f2{s}", tag="f2")
                u1 = pool.tile([NPART, HP], f16, name=f"u1{s}", tag="u1")
                u2 = pool.tile([NPART, HP], f16, name=f"u2{s}", tag="u2")
                u2b = pool.tile([NPART, HP], f16, name=f"u2b{s}", tag="u2b")
                s2 = pool.tile([NPART, HP + 1], f16, name=f"s2_{s}", tag="s2")
                yy = pool.tile([NPART, 2 * (HP + 1)], f16, name=f"yy{s}", tag="yy")

                # ---- input DMAs (order = earliest consumer first; the
                # first few are single-plane so DVE/Pool start ASAP) ----
                def y_dma(pl, n):
                    nc.sync.dma_start(
                        yt[:, pl * YW : (pl + n) * YW],
                        _dram_view(
                            yg,
                            (r0 * CH * 2 + pl) * YW,
                            [(CH * 2 * YW, RS), (2 * YW, CH), (1, n * YW)],
                        ),
                    )

                if s == 0:
                    # slab 0: feed DVE's FIR first, then Pool's planes
                    y_dma(1, 1)                   # yo
                    in_dma(a1t[:, 0:HP], r0, 0, 1)            # a1
                    y_dma(0, 1)                   # ye
                    in_dma(a1t[:, HP : 3 * HP], r0, 1, 2)     # a2 a3
                    if ORDER0 == "ac":
                        in_dma(a2t[:, :], r0, 6, 3)   # a4 a5 a6
                        in_dma(c1t[:, :], r0, 3, 3)   # c0 c1 c2
                    else:
                        in_dma(c1t[:, :], r0, 3, 3)   # c0 c1 c2
                        in_dma(a2t[:, :], r0, 6, 3)   # a4 a5 a6
                    in_dma(c2t[:, :], r0, 9, 4)   # c3 c4 c5 c6
                    in_dma(bt[:, :], r0, 13, 4)   # b1e b2e e10 e11
                else:
                    # later slabs: Pool resumes first (DVE is still busy
                    # with the previous slab's chain), so c-planes lead
                    in_dma(c1t[:, :], r0, 3, 3)   # c0 c1 c2
                    y_dma(0, 2)                   # ye yo
                    in_dma(a1t[:, :], r0, 0, 3)   # a1 a2 a3
                    in_dma(a2t[:, :], r0, 6, 3)   # a4 a5 a6
                    in_dma(c2t[:, :], r0, 9, 4)   # c3 c4 c5 c6
                    in_dma(bt[:, :], r0, 13, 4)   # b1e b2e e10 e11

                # plane views
                def yev(d):  # ye[j - d]
                    return yt[:, HALO - d : HALO - d + HP]

                def yov(d):  # yo[j - d]
                    return yt[:, YW + HALO - d : YW + HALO - d + HP]

                def a1v(k):
                    return a1t[:, k * HP : (k + 1) * HP]

                def a2v(k):
                    return a2t[:, k * HP : (k + 1) * HP]

                def c1v(k):
                    return c1t[:, k * HP : (k + 1) * HP]

                def c2v(k):
                    return c2t[:, k * HP : (k + 1) * HP]

                b1e = bt[:, 0:HP]
                b2e = bt[:, HP : 2 * HP]
                e10 = bt[:, 2 * HP : 3 * HP]
                e11 = bt[:, 3 * HP : 4 * HP]

                # xe has a 1-elem zero guard so sh(xe) reads are in-tile
                xeg = xe[:, 0:HP]     # xe[m-1] (shifted) view
                xeb = xe[:, 1 : HP + 1]  # xe[m] view

                # ---- scan/shift guards ----
                v.memset(xe[:, 0:1], 0.0)
                v.memset(s2[:, 0:1], 0.0)
                if VARIANT == "a2x":
                    v.memset(yy[:, 0:1], 0.0)          # yed[0] (sh read)
                else:
                    v.memset(yy[:, HP + 1 : HP + 2], 0.0)  # yod[0] (sh read)

                # ---- f2 tap list: f2 = yo + sum_k c_k * y_shift_k ----
                # taps c0..c2 come from c1t, c3..c6 from c2t.
                f2taps = [
                    (c1v(0), yev(0)),
                    (c1v(1), yov(1)),
                    (c1v(2), yev(1)),
                    (c2v(0), yov(2)),
                    (c2v(1), yev(2)),
                    (c2v(2), yov(3)),
                    (c2v(3), yev(3)),
                ]
                # Pool executes the first POOL_OPS[s] ops of the flat op list
                # [mul0, add_base, mul1, add1, mul2, add2, mul3, add3]:
                # an odd count means the last pool op is a mul whose product
                # (in pt) is folded in by DVE.
                P = POOL_OPS[s]
                ntap_pool = (P + 1) // 2  # taps pool multiplies
                pool_tail_mul = P % 2 == 1
                g.tensor_mul(pf[:], *f2taps[0])
                g.tensor_add(pf[:], pf[:], yov(0))
                for k in range(1, ntap_pool):
                    g.tensor_mul(pt[:], *f2taps[k])
                    if 2 * (k + 1) <= P:
                        g.tensor_add(pf[:], pf[:], pt[:])

                # ---- DVE: xe FIR ----
                v.tensor_mul(xeb, a1v(0), yov(1))
                v.tensor_add(xeb, xeb, yev(0))
                v.tensor_mul(tv[:], a1v(1), yev(1))
                v.tensor_add(xeb, xeb, tv[:])
                v.tensor_mul(tv[:], a1v(2), yov(2))
                v.tensor_add(xeb, xeb, tv[:])
                v.tensor_mul(tv[:], a2v(0), yev(2))
                v.tensor_add(xeb, xeb, tv[:])
                v.tensor_mul(tv[:], a2v(1), yov(3))
                v.tensor_add(xeb, xeb, tv[:])
                v.tensor_mul(tv[:], a2v(2), yev(3))
                v.tensor_add(xeb, xeb, tv[:])

                # ---- DVE: f2 tail (remaining taps) + merge with Pool ----
                v.tensor_mul(qv[:], *f2taps[ntap_pool])
                if pool_tail_mul:
                    v.tensor_add(qv[:], qv[:], pt[:])
                for k in range(ntap_pool + 1, 7):
                    v.tensor_mul(tv[:], *f2taps[k])
                    v.tensor_add(qv[:], qv[:], tv[:])
                v.tensor_add(f2[:], qv[:], pf[:])

                yed = yy[:, 0 : HP + 1]
                yod = yy[:, HP + 1 : 2 * (HP + 1)]

                def tts(out2, d0, d1):
                    v.tensor_tensor_scan(out2, d0, d1, 0.0, MULT, ADD)

                def out_dma(plane, p):
                    nc.sync.dma_start(
                        _dram_view(
                            out_d,
                            (r0 * 2 + p) * CH * HL,
                            [(2 * CH * HL, RS), (HL, CH), (1, HL)],
                        ),
                        plane[:, 1 + W // 2 : 1 + W // 2 + HL],
                    )

                if VARIANT == "a2x":
                    # ---- GS A2x: s1^0 := xe, then a full sweep ----
                    v.tensor_mul(u2[:], e10, xeg)
                    v.tensor_add(u2[:], u2[:], f2[:])
                    tts(s2[:, 1:], e11, u2[:])
                    v.tensor_mul(u1[:], b1e, s2[:, 0:HP])
                    v.tensor_add(u1[:], u1[:], xeb)
                    tts(yed[:, 1:], b2e, u1[:])
                    v.tensor_mul(u2b[:], e10, yed[:, 0:HP])
                    v.tensor_add(u2b[:], u2b[:], f2[:])
                    out_dma(yed, 0)
                    if s < NSLAB - 1:
                        tts(yod[:, 1:], e11, u2b[:])
                        out_dma(yod, 1)
                    else:
                        # last slab: split the final scan so the first
                        # half's output DMA overlaps the second half
                        HH = HP // 2
                        tts(yod[:, 1 : 1 + HH], e11[:, 0:HH], u2b[:, 0:HH])
                        nc.sync.dma_start(
                            _dram_view(
                                out_d,
                                (r0 * 2 + 1) * CH * HL,
                                [(2 * CH * HL, RS), (HL, CH), (1, HH - W // 2)],
                            ),
                            yod[:, 1 + W // 2 : 1 + HH],
                        )
                        v.tensor_tensor_scan(
                            yod[:, 1 + HH : 1 + HP],
                            e11[:, HH:HP],
                            u2b[:, HH:HP],
                            yod[:, HH : HH + 1],
                            MULT,
                            ADD,
                        )
                        nc.sync.dma_start(
                            _dram_view(
                                out_d,
                                (r0 * 2 + 1) * CH * HL + (HH - W // 2),
                                [(2 * CH * HL, RS), (HL, CH), (1, HP - HH)],
                            ),
                            yod[:, 1 + HH : 1 + HP],
                        )
                else:
                    # ---- GS Axy: s1^0 := xe + b2e*sh(xe), odd, even ----
                    # (s2 tile reused for the Neumann start s1h)
                    v.tensor_mul(tv[:], b2e, xeg)
                    v.tensor_add(s2[:, 1:], tv[:], xeb)
                    v.tensor_mul(u2[:], e10, s2[:, 0:HP])
                    v.tensor_add(u2[:], u2[:], f2[:])
                    tts(yod[:, 1:], e11, u2[:])
                    v.tensor_mul(u1[:], b1e, yod[:, 0:HP])
                    v.tensor_add(u1[:], u1[:], xeb)
                    out_dma(yod, 1)
                    tts(yed[:, 1:], b2e, u1[:])
                    out_dma(yed, 0)

    nc.compile()
    return nc


def _prep_inputs(y, A_exc, A_loop):
    y = np.asarray(y, dtype=np.float32)
    A_exc = np.asarray(A_exc, dtype=np.float32)
    A_loop = np.asarray(A_loop, dtype=np.float32)

    TP = PRE + T
    y_pad = np.zeros((B, TP), np.float32)
    y_pad[:, PRE:] = y
    b1 = np.zeros((B, TP), np.float32)
    b2 = np.zeros((B, TP), np.float32)
    b1[:, PRE:] = -A_loop[:, :, 0]
    b2[:, PRE:] = -A_loop[:, :, 1]
    a_pad = np.zeros((B, TP, 6), np.float32)
    a_pad[:, PRE:, :] = A_exc

    # half-grid planes (length TPh)
    ye = y_pad[:, 0::2]
    yo = y_pad[:, 1::2]
    ae = [np.ascontiguousarray(a_pad[:, 0::2, k]) for k in range(6)]
    ao = [np.ascontiguousarray(a_pad[:, 1::2, k]) for k in range(6)]
    b1e, b1o = b1[:, 0::2], b1[:, 1::2]
    b2e, b2o = b2[:, 0::2], b2[:, 1::2]

    e10 = b1o * b2e
    e11 = b1o * b1e + b2o
    c = [
        b1o + ao[0],
        b1o * ae[0] + ao[1],
        b1o * ae[1] + ao[2],
        b1o * ae[2] + ao[3],
        b1o * ae[3] + ao[4],
        b1o * ae[4] + ao[5],
        b1o * ae[5],
    ]

    # full-row plane stacks, fp16
    # order: a1 a2 a3 | c0 c1 c2 | a4 a5 a6 | c3 c4 c5 c6 | b1e b2e e10 e11
    pl = np.stack(
        [ae[0], ae[1], ae[2], c[0], c[1], c[2], ae[3], ae[4], ae[5],
         c[3], c[4], c[5], c[6], b1e, b2e, e10, e11],
        axis=1,
    ).astype(np.float16)          # (B, NPL, TPh)
    yl = np.stack([ye, yo], axis=1).astype(np.float16)  # (B, 2, TPh)

    # per-chunk gather (duplicates the warmup halo; chunk c starts at
    # half-grid index HALO + c*HL for coefficients, c*HL for y-with-halo)
    st_p = pl.strides
    pg = np.lib.stride_tricks.as_strided(
        pl[:, :, HALO:],
        shape=(B, CH, NPL, HP),
        strides=(st_p[0], HL * st_p[2], st_p[1], st_p[2]),
    )
    st_y = yl.strides
    ygs = np.lib.stride_tricks.as_strided(
        yl,
        shape=(B, CH, 2, YW),
        strides=(st_y[0], HL * st_y[2], st_y[1], st_y[2]),
    )

    in_maps = []
    for cix in range(NCORES):
        r0, r1 = cix * BLOC, (cix + 1) * BLOC
        in_maps.append(
            {
                "yg": np.ascontiguousarray(ygs[r0:r1]),
                "pg": np.ascontiguousarray(pg[r0:r1]),
            }
        )
    return in_maps


def _get_program():
    if "nc" not in _compiled:
        _compiled["nc"] = _build_program()
    return _compiled["nc"]


def run(y, A_exc, A_loop, trace=False, **trace_kwargs):
    """Returns (output, BassKernelResults)."""
    nc = _get_program()
    in_maps = _prep_inputs(y, A_exc, A_loop)
    res = run_bass_kernel_spmd(
        nc, in_maps, list(range(NCORES)), trace=trace, **trace_kwargs
    )
    out = np.empty((B, T), np.float32)
    for cix in range(NCORES):
        o = res.results[cix]["y_out"].astype(np.float32)  # (BLOC, 2, CH*HL)
        blk = out[cix * BLOC : (cix + 1) * BLOC]
        blk[:, 0::2] = o[:, 0, :]
        blk[:, 1::2] = o[:, 1, :]
    return out, res


def kernel(y, A_exc, A_loop):
    out, _ = run(y, A_exc, A_loop)
    return out
